# revision 5
# baseline (speedup 1.0000x reference)
"""CNF (continuous normalizing flow) RK4 kernel for 8 Trainium2 NeuronCores.

Computes z, log_det = RK4-integrate of
    dx/dt = f(x,t) = W3.T-style MLP(x,t),  d(log_det)/dt = -trace(df/dx)
over t in [0,1] with 8 fixed steps, matching reference.py.

Key algebra: the exact Jacobian trace of the MLP collapses to
    tr = g1^T (W2 * C^T) g2,   C = W3 @ W1[:D],  g_i = 1 - h_i^2
so no [B,D,D] Jacobian is ever materialized -- one extra HxH matmul per
RK stage ("trace matmul") replaces the jacfwd.

Sharding: pure data parallel, batch 512 -> 8 cores x 64 rows.

Per-core dataflow (per RK stage):
  u^T [66,B] stationary --W1e moving--> a1 [B,512] (PSUM, fp32r matmul)
  tanh -> h1 ; PE-transpose -> h1T [512,B] chunks (stationary for L2)
  h1T --W2 moving--> a2 [B,512] ; tanh -> h2 (bf16) ; PE-transpose -> h2T
  g2T = 1-h2T^2 ; trace: g2T stationary --MT moving--> y [B,512]
  ld += reduce( (s1-1)*w * y )  via fused tensor_tensor_reduce
  L3: W3-prescaled (RK4 coeffs folded in) stationary, h2T moving ->
      dxT increments accumulated directly in PSUM (S per-stage, F per-step)
"""

import numpy as np

_B, _D, _H = 512, 64, 512
_NSTEPS = 8
_NCORES = 8
_BL = _B // _NCORES  # 64 batch rows per core
_DT = 1.0 / _NSTEPS

_compiled = {}


def _build_nc(use_bf16, with_b1, with_b2, with_b3, reps):
    import concourse.bacc as bacc
    import concourse.mybir as mybir
    import concourse.tile as tile

    dt = mybir.dt
    f32, f32r, bf16 = dt.float32, dt.float32r, dt.bfloat16
    tdt = bf16 if use_bf16 else f32r  # trace-path / L3 dtype
    AF = mybir.ActivationFunctionType
    OP = mybir.AluOpType

    D, H, BL = _D, _H, _BL
    KU = D + 1  # u = [x, t]
    NCH = H // 128  # 4 k-chunks

    # per-stage RK coefficients
    S_VAR = [0, 0, 1]            # W3 scale variant for state increments (dt/2, dt/2, dt)
    F_VAR = [2, 3, 3, 2]         # W3 scale variant for final sum (dt/6, dt/3, dt/3, dt/6)
    LD_W = [_DT / 6.0, _DT / 3.0, _DT / 3.0, _DT / 6.0]

    nc = bacc.Bacc("TRN2", target_bir_lowering=False, debug=False,
                   num_devices=_NCORES)

    u0_d = nc.dram_tensor("u0", [KU, BL], f32r, kind="ExternalInput")
    W1e_d = nc.dram_tensor("W1e", [KU, H], f32r, kind="ExternalInput")
    if with_b1:
        b1r_d = nc.dram_tensor("b1r", [1, H], f32r, kind="ExternalInput")
    W2s_d = nc.dram_tensor("W2s", [128, NCH, H], f32r, kind="ExternalInput")
    MTs_d = nc.dram_tensor("MTs", [128, NCH, H], tdt, kind="ExternalInput")
    W3p_d = nc.dram_tensor("W3p", [128, NCH, 4, D], tdt, kind="ExternalInput")
    idr_d = nc.dram_tensor("idr", [BL, BL], f32r, kind="ExternalInput")
    trows_d = nc.dram_tensor("trows", [1, 17 * BL], f32r, kind="ExternalInput")
    if use_bf16:
        idb_d = nc.dram_tensor("idb", [BL, BL], tdt, kind="ExternalInput")
    if with_b2:
        b2r_d = nc.dram_tensor("b2r", [1, H], f32r, kind="ExternalInput")
    if with_b1 or with_b2:
        onesr_d = nc.dram_tensor("onesr", [1, BL], f32r, kind="ExternalInput")
    if with_b3:
        b3p_d = nc.dram_tensor("b3p", [1, 4, D], tdt, kind="ExternalInput")
        onesb_d = nc.dram_tensor("onesb", [1, BL], tdt, kind="ExternalInput")
    zT_d = nc.dram_tensor("zT", [D, BL], f32r, kind="ExternalOutput")
    ldo_d = nc.dram_tensor("ldo", [BL, 1], f32, kind="ExternalOutput")

    with tile.TileContext(nc) as tc:
        with (
            tc.tile_pool(name="const", bufs=1) as cp,
            tc.tile_pool(name="state", bufs=2) as sp,
            tc.tile_pool(name="work", bufs=2) as wp,
            tc.tile_pool(name="pa", bufs=2, space="PSUM") as pa,
            tc.tile_pool(name="pt", bufs=1, space="PSUM") as pt,
            tc.tile_pool(name="psf", bufs=2, space="PSUM") as psf,
        ):
            # ---- constants ----
            W1e = cp.tile([KU, H], f32r)
            nc.sync.dma_start(W1e[:], W1e_d[:])
            W2s = cp.tile([128, NCH, H], f32r)
            nc.sync.dma_start(W2s[:], W2s_d[:])
            MTs = cp.tile([128, NCH, H], tdt)
            nc.sync.dma_start(MTs[:], MTs_d[:])
            W3p = cp.tile([128, NCH, 4, D], tdt)
            nc.sync.dma_start(W3p[:], W3p_d[:])
            idr = cp.tile([BL, BL], f32r)
            nc.sync.dma_start(idr[:], idr_d[:])
            trows = cp.tile([1, 17 * BL], f32r)
            nc.sync.dma_start(trows[:], trows_d[:])
            if use_bf16:
                idb = cp.tile([BL, BL], tdt)
                nc.sync.dma_start(idb[:], idb_d[:])
            else:
                idb = idr
            if with_b1:
                b1r = cp.tile([1, H], f32r)
                nc.sync.dma_start(b1r[:], b1r_d[:])
            if with_b2:
                b2r = cp.tile([1, H], f32r)
                nc.sync.dma_start(b2r[:], b2r_d[:])
            if with_b1 or with_b2:
                onesr = cp.tile([1, BL], f32r)
                nc.sync.dma_start(onesr[:], onesr_d[:])
            if with_b3:
                b3p = cp.tile([1, 4, D], tdt)
                nc.sync.dma_start(b3p[:], b3p_d[:])
                onesb = cp.tile([1, BL], tdt)
                nc.sync.dma_start(onesb[:], onesb_d[:])
            zb = cp.tile([128, 1], f32)
            nc.gpsimd.memset(zb[:], 0.0)

            # ---- state ----
            ubase = sp.tile([KU, BL], f32r, tag="ubase")
            nc.sync.dma_start(ubase[:], u0_d[:])
            ld = sp.tile([BL, 1], f32, tag="ld")
            nc.vector.memset(ld[:], 0.0)

            def stage(u_i, t_idx, i, pF, first_f, last_f):
                """One aug_dyn evaluation. Returns pS (or None)."""
                nonlocal ld
                # t row of u (copy from host-side constant table)
                nc.vector.tensor_copy(u_i[D:D + 1, :],
                                      trows[0:1, t_idx * BL:(t_idx + 1) * BL])

                # L1: a1 = u @ W1 (+ b1)
                a1 = pa.tile([BL, H], f32, tag="a")
                nc.tensor.matmul(a1[:], u_i[:], W1e[:], start=True,
                                 stop=not with_b1)
                if with_b1:
                    nc.tensor.matmul(a1[:], onesr[:], b1r[:],
                                     start=False, stop=True)
                h1 = wp.tile([BL, H], f32r, tag="h1")
                nc.scalar.activation(h1[:], a1[:], AF.Tanh,
                                     bias=zb[0:BL, :])

                # transpose h1 -> h1T [128, NCH, BL] in two waves
                h1T = wp.tile([128, NCH, BL], f32r, tag="h1T")
                for w in range(2):
                    tp = pt.tile([128, 2, 512], f32r, tag="tp")
                    for jj in range(2):
                        j = 2 * w + jj
                        nc.tensor.transpose(tp[:, jj, 0:BL],
                                            h1[:, j * 128:(j + 1) * 128],
                                            idr[:])
                    nc.vector.tensor_copy(
                        h1T[:, 2 * w:2 * w + 2, :], tp[:, :, 0:BL])

                # L2: a2 = h1 @ W2 (+ b2)
                a2 = pa.tile([BL, H], f32, tag="a")
                for j in range(NCH):
                    nc.tensor.matmul(a2[:], h1T[:, j, :], W2s[:, j, :],
                                     start=(j == 0),
                                     stop=(j == NCH - 1 and not with_b2))
                if with_b2:
                    nc.tensor.matmul(a2[:], onesr[:], b2r[:],
                                     start=False, stop=True)

                # trace g1 path: g1w = (h1^2 - 1) * w_i   (= -w_i * g1)
                s1 = wp.tile([BL, H], f32, tag="s1")
                nc.scalar.activation(s1[:], h1[:], AF.Square,
                                     bias=zb[0:BL, :])
                g1w = wp.tile([BL, H], f32, tag="g1w")
                nc.vector.tensor_scalar(g1w[:], s1[:], 1.0, float(LD_W[i]),
                                        OP.subtract, OP.mult)

                # h2 (trace dtype), transpose -> h2T
                h2 = wp.tile([BL, H], tdt, tag="h2")
                nc.scalar.activation(h2[:], a2[:], AF.Tanh, bias=zb[0:BL, :])
                h2T = wp.tile([128, NCH, BL], tdt, tag="h2T")
                tpw = 1024 if use_bf16 else 512
                for w in range(2):
                    tp = pt.tile([128, 2, tpw], tdt, tag="tp")
                    for jj in range(2):
                        j = 2 * w + jj
                        nc.tensor.transpose(tp[:, jj, 0:BL],
                                            h2[:, j * 128:(j + 1) * 128],
                                            idb[:])
                    nc.vector.tensor_copy(h2T[:, 2 * w:2 * w + 2, :],
                                          tp[:, :, 0:BL])

                # g2T = 1 - h2T^2  (as (h2T^2 - 1) * -1)
                s2T = wp.tile([128, NCH, BL], tdt, tag="s2T")
                nc.vector.tensor_mul(s2T[:], h2T[:], h2T[:])
                g2T = wp.tile([128, NCH, BL], tdt, tag="g2T")
                nc.vector.tensor_scalar(g2T[:], s2T[:], 1.0, -1.0,
                                        OP.subtract, OP.mult)

                # trace matmul: y = g2 @ M^T
                y = pa.tile([BL, H], f32, tag="a")
                for j in range(NCH):
                    nc.tensor.matmul(y[:], g2T[:, j, :], MTs[:, j, :],
                                     start=(j == 0), stop=(j == NCH - 1))

                # ld_new = ld + sum(g1w * y)   [= ld - w_i * trace]
                scr = wp.tile([BL, H], f32, tag="scr")
                nc.vector.tensor_mul(scr[:], g1w[:], y[:])
                scr2 = wp.tile([BL, H], f32, tag="scr2")
                tr = wp.tile([BL, 1], f32, tag="tr")
                nc.scalar.activation(scr2[:], scr[:], AF.Identity,
                                     bias=zb[0:BL, :], accum_out=tr[:])
                ld_new = sp.tile([BL, 1], f32, tag="ld")
                nc.gpsimd.tensor_tensor(ld_new[:], ld[:], tr[:], OP.add)
                ld = ld_new

                # L3 state increment (stages 0-2): pS = c_i * (W3^T h2 + b3)
                pS = None
                if i < 3:
                    sv = S_VAR[i]
                    pS = psf.tile([D, BL], f32, tag="S")
                    for j in range(NCH):
                        nc.tensor.matmul(pS[:], W3p[:, j, sv, :], h2T[:, j, :],
                                         start=(j == 0),
                                         stop=(j == NCH - 1 and not with_b3))
                    if with_b3:
                        nc.tensor.matmul(pS[:], b3p[0:1, sv, :], onesb[:],
                                         start=False, stop=True)
                # L3 final-sum accumulation: pF += d_i * (W3^T h2 + b3)
                fv = F_VAR[i]
                for j in range(NCH):
                    nc.tensor.matmul(pF[:], W3p[:, j, fv, :], h2T[:, j, :],
                                     start=(first_f and j == 0),
                                     stop=(last_f and j == NCH - 1
                                           and not with_b3))
                if with_b3:
                    nc.tensor.matmul(pF[:], b3p[0:1, fv, :], onesb[:],
                                     start=False, stop=last_f)
                return pS

            for _rep in range(reps):
                for s in range(_NSTEPS):
                    pF = psf.tile([D, BL], f32, tag="F")
                    u_i = ubase
                    for i in range(4):
                        pS = stage(u_i, 2 * s + (0, 1, 1, 2)[i], i, pF,
                                   first_f=(i == 0), last_f=(i == 3))
                        if i < 3:
                            u_n = wp.tile([KU, BL], f32r, tag="ust")
                            nc.vector.tensor_tensor(
                                u_n[0:D, :], ubase[0:D, :], pS[:], OP.add)
                            u_i = u_n
                    ub_new = sp.tile([KU, BL], f32r, tag="ubase")
                    nc.vector.tensor_tensor(
                        ub_new[0:D, :], ubase[0:D, :], pF[:], OP.add)
                    ubase = ub_new

            nc.sync.dma_start(zT_d[:], ubase[0:D, :])
            nc.sync.dma_start(ldo_d[:], ld[:])

    nc.compile()
    return nc


def _get_nc(use_bf16, with_b1, with_b2, with_b3, reps=1):
    key = (use_bf16, with_b1, with_b2, with_b3, reps)
    if key not in _compiled:
        _compiled[key] = _build_nc(*key)
    return _compiled[key]


def _host_inputs(x, W1, b1, W2, b2, W3, b3, use_bf16, with_b1, with_b2, with_b3):
    import ml_dtypes
    f32 = np.float32
    tnp = ml_dtypes.bfloat16 if use_bf16 else f32
    D, H, BL = _D, _H, _BL
    NCH = H // 128

    W1_64 = W1.astype(np.float64)
    W3_64 = W3.astype(np.float64)
    C = W3_64 @ W1_64[:D]                      # [H, H]
    MT = (W2.astype(np.float64).T * C)         # MT[k2,k1] = W2[k1,k2]*C[k2,k1]

    W1e = W1.astype(f32)                                          # [65, H]
    W2s = np.ascontiguousarray(
        W2.reshape(NCH, 128, H).transpose(1, 0, 2)).astype(f32)
    MTs = np.ascontiguousarray(
        MT.reshape(NCH, 128, H).transpose(1, 0, 2)).astype(tnp)
    scales = [_DT / 2, _DT, _DT / 6, _DT / 3]
    W3v = np.stack([(sc * W3.astype(np.float64)).reshape(NCH, 128, D)
                    for sc in scales], axis=0)           # [4v, NCH, 128, D]
    W3p = np.ascontiguousarray(W3v.transpose(2, 1, 0, 3)).astype(tnp)
    idr = np.eye(BL, dtype=f32)
    tvals = np.arange(17, dtype=np.float64) * (_DT / 2)
    trows = np.repeat(tvals, BL)[None, :].astype(f32)

    shared = {"W1e": W1e, "W2s": W2s, "MTs": MTs, "W3p": W3p, "idr": idr,
              "trows": trows}
    if use_bf16:
        shared["idb"] = np.eye(BL, dtype=tnp)
    if with_b1:
        shared["b1r"] = b1[None, :].astype(f32)
    if with_b2:
        shared["b2r"] = b2[None, :].astype(f32)
    if with_b1 or with_b2:
        shared["onesr"] = np.ones((1, BL), f32)
    if with_b3:
        b3v = np.stack([(sc * b3.astype(np.float64)) for sc in scales], 0)
        shared["b3p"] = np.ascontiguousarray(b3v[None, :, :]).astype(tnp)
        shared["onesb"] = np.ones((1, BL), tnp)

    xs = x.reshape(_NCORES, BL, D)
    in_maps = []
    for c in range(_NCORES):
        u0 = np.zeros((D + 1, BL), f32)
        u0[0:D] = xs[c].T
        in_maps.append({"u0": u0, **shared})
    return in_maps


def run(x, W1, b1, W2, b2, W3, b3, use_bf16=True, reps=1):
    from concourse.bass_utils import run_bass_kernel_spmd
    x = np.asarray(x, np.float32)
    W1 = np.asarray(W1, np.float32)
    b1 = np.asarray(b1, np.float32)
    W2 = np.asarray(W2, np.float32)
    b2 = np.asarray(b2, np.float32)
    W3 = np.asarray(W3, np.float32)
    b3 = np.asarray(b3, np.float32)
    with_b1 = bool(np.any(b1 != 0.0))
    with_b2 = bool(np.any(b2 != 0.0))
    with_b3 = bool(np.any(b3 != 0.0))
    nc = _get_nc(use_bf16, with_b1, with_b2, with_b3, reps)
    in_maps = _host_inputs(x, W1, b1, W2, b2, W3, b3,
                           use_bf16, with_b1, with_b2, with_b3)
    res = run_bass_kernel_spmd(nc, in_maps, core_ids=list(range(_NCORES)))
    z = np.concatenate([r["zT"].T for r in res.results], axis=0)
    log_det = np.concatenate([r["ldo"][:, 0] for r in res.results], axis=0)
    return z.astype(np.float32), log_det.astype(np.float32)


def kernel(x, W1, b1, W2, b2, W3, b3):
    return run(x, W1, b1, W2, b2, W3, b3)
